# revision 44
# baseline (speedup 1.0000x reference)
"""Multi-head self-attention (B=2, S=2048, D=1024, H=16) on 8 TRN2 NeuronCores.

Sharding: batch*heads tensor-parallel. Each core owns 2 heads (both batches):
QKV projection for its heads only (W_qkv output-dim sharded), full attention
for its 2x2 (batch, head) pairs, partial output projection (W_out input-dim
sharded). The 8 bf16 partial outputs are summed in fp64 on the host (the
"all-reduce"), plus the output bias.

Single fused pipeline, PE-saturated (~90% tensor-engine occupancy):
  - All matmuls bf16 (FWL fast-weight-load); x ships pre-transposed bf16
    (xT [D, T]); per-core weight slices pre-transposed on host.
  - q,k head-major (qT/kT [hd 128, tok]). v is produced DIRECTLY token-major
    by swapping matmul operands (lhsT = x tile, rhs = Wv) so no PE
    transposes; v tiles carry a LEADING 64-wide ones block per head
    (va [128, 2, 128] = [ones | v]) so each AV matmul accumulates the
    softmax denominator already BROADCAST across psum rows 0..63 (output
    in rows 64..127) — free, since matmul cost is column-count-bound.
  - attention runs in 8 loops of (batch, 512-token q-chunk), 8 k-steps each.
    Scores are k-token-major with TWO k-tiles packed per psum tile
    [128, 1024] (ki pair = column halves): exp stays 1024-wide (ACT
    efficiency) while the AV accumulators stay [128, 512] x 2 heads
    = 2 psum banks. No max-subtraction (|score*scale| < ~6 here).
    PSUM budget: scores 2x2 banks + accs 2 banks + 2 "fill" banks.
  - The 2 fill banks host everything else via tag round-robin: QKV
    projection accumulators, output-projection tiles, and the normalization
    broadcast — emitted as FILLER closures pumped between k-steps so the PE
    queue never drains: batch 1's projections run inside batch 0's attention
    loops, output projections run inside later loops. ACT runs exp plus
    the QKV-projection evacuations (early, where it idles); output
    evacuations are DVE.
  - normalization per (loop, head): reciprocal_approx_fast straight on the
    psum denominator block (rows 0..63 — the custom op corrupts on psum
    reads at base partition 64 and NaNs on 1-partition rows), one DVE
    multiply vs psum o-rows -> oT bf16. No evacuation, no broadcast matmul.
  - first x chunk DMA'd in t-quarters so the first projection matmuls start
    as soon as the first 256KB lands; NEURON_RT_RESET_CORES=1 restores
    nominal clocks if a previous run degraded the device power state.
"""

import os
import sys

# Reset cores at runtime init: restores nominal clocks if a previous run
# left the device in a degraded power state (observed ~20% slowdown).
os.environ.setdefault("NEURON_RT_RESET_CORES", "1")

for _p in ("/opt/trn_rl_repo", "/root/.axon_site/_ro/trn_rl_repo"):
    if _p not in sys.path:
        sys.path.insert(0, _p)

from collections import deque
from contextlib import ExitStack

import numpy as np

import concourse.bacc as bacc
import concourse.bass as bass
import concourse.mybir as mybir
import concourse.tile as tile
from concourse.bass_utils import run_bass_kernel_spmd

F32 = mybir.dt.float32
F32R = mybir.dt.float32r
BF16 = mybir.dt.bfloat16

B, S, D, H = 2, 2048, 1024, 16
HD = D // H  # 64
T = B * S  # 4096 tokens
SCALE = HD**-0.5
N_CORES = 8
HEADS_PER_CORE = H // N_CORES  # 2

EXP = mybir.ActivationFunctionType.Exp


def build_kernel() -> bacc.Bacc:
    nc = bacc.Bacc(target_bir_lowering=False)
    xT = nc.dram_tensor("xT", [D, T], BF16, kind="ExternalInput")
    wqkvT = nc.dram_tensor("wqkvT", [D, 384], BF16, kind="ExternalInput")
    woutT = nc.dram_tensor("woutT", [2 * HD, D], BF16, kind="ExternalInput")
    out = nc.dram_tensor("out", [T, D], BF16, kind="ExternalOutput")

    with tile.TileContext(nc) as tc, ExitStack() as ctx:
        const = ctx.enter_context(tc.tile_pool(name="const", bufs=1))
        sb = ctx.enter_context(tc.tile_pool(name="sb", bufs=1))
        ps = ctx.enter_context(tc.tile_pool(name="ps", bufs=1, space="PSUM"))

        w_sb = const.tile([128, 8, 384], BF16)
        nc.sync.dma_start(out=w_sb, in_=wqkvT.rearrange("(t p) c -> p t c", p=128))
        wo = const.tile([2 * HD, D], BF16)
        nc.sync.dma_start(out=wo, in_=woutT[:, :])

        # ---- x prefetch: all 8 chunks (b-major so b=1 arrives during b=0) --
        x_sb = {}
        for b in range(B):
            for ch in range(4):
                t = sb.tile([128, 8, 512], BF16, tag="x", bufs=8, name=f"x{b}{ch}")
                tok0 = b * S + ch * 512
                xsrc = xT[:, tok0 : tok0 + 512].rearrange("(t p) n -> p t n", p=128)
                if b == 0 and ch == 0:
                    for tq in range(4):
                        nc.sync.dma_start(
                            out=t[:, 2 * tq : 2 * tq + 2, :],
                            in_=xsrc[:, 2 * tq : 2 * tq + 2, :],
                        )
                else:
                    nc.sync.dma_start(out=t, in_=xsrc)
                x_sb[b, ch] = t

        qT, kT, oT, va3 = {}, {}, {}, {}
        for b in range(B):
            qT[b] = sb.tile([128, S], BF16, tag="qk", bufs=4, name=f"qT{b}")
            kT[b] = sb.tile([128, S], BF16, tag="qk", bufs=4, name=f"kT{b}")
            oT[b] = sb.tile([128, S], BF16, tag="ot", bufs=2, name=f"oT{b}")
            va3[b] = [None] * 16

        # ---------------- filler machinery ----------------
        fillers = deque()

        def pump(n):
            c = 0
            while fillers and c < n:
                fillers.popleft()()
                c += 1

        def qk_unit(b, ch, g):
            """QKV projection for q (g=0) or k (g=1): 8 accumulating matmuls
            + DVE evacuation into qT/kT."""

            def go():
                acc = ps.tile([128, 512], F32, tag="fill", bufs=2, name="qkacc")
                xs = x_sb[b, ch]
                for t in range(8):
                    nc.tensor.matmul(
                        acc[:],
                        w_sb[:, t, g * 128 : (g + 1) * 128],
                        xs[:, t, :],
                        start=(t == 0),
                        stop=(t == 7),
                    )
                dst = qT[b] if g == 0 else kT[b]
                nc.scalar.copy(dst[:, ch * 512 : (ch + 1) * 512], acc[:])

            return go

        def v_unit(b, ch, tt):
            """v projection for one 128-token tile, produced token-major
            (lhsT = x chunk, rhs = Wv), then augmented with ones columns."""

            def go():
                vt = ps.tile([128, 2, 64], F32, tag="fill", bufs=2, name="vt")
                xs = x_sb[b, ch]
                for t in range(8):
                    nc.tensor.matmul(
                        vt[:, :, :],
                        xs[:, t, tt * 128 : (tt + 1) * 128],
                        w_sb[:, t, 256:384],
                        start=(t == 0),
                        stop=(t == 7),
                    )
                va = sb.tile([128, 2, 128], BF16, tag="vaug", bufs=32, name="va")
                nc.vector.memset(va[:, :, 0:64], 1.0)
                nc.vector.tensor_copy(va[:, :, 64:128], vt[:, :, :])
                va3[b][ch * 4 + tt] = va

            return go

        def op_unit(b, tc_i, dve_only=False):
            """Output projection for one 128-token chunk: 2 matmuls (dout
            halves) + evac (DVE, or split DVE/ACT when ACT has slack) + DMA."""

            def go():
                for nk in range(2):
                    op = ps.tile([128, 512], F32, tag="fill", bufs=2, name="opps")
                    nc.tensor.matmul(
                        op[:],
                        oT[b][:, tc_i * 128 : (tc_i + 1) * 128],
                        wo[:, nk * 512 : (nk + 1) * 512],
                        start=True,
                        stop=True,
                    )
                    ob = sb.tile([128, 512], BF16, tag="outsb", bufs=10, name="ob")
                    if nk == 0 or dve_only:
                        nc.vector.tensor_copy(ob[:], op[:])
                    else:
                        nc.scalar.copy(ob[:], op[:])
                    r0 = b * S + tc_i * 128
                    nc.sync.dma_start(
                        out=out[r0 : r0 + 128, nk * 512 : (nk + 1) * 512], in_=ob[:]
                    )

            return go

        # ---------------- P1(b0): k/v ch0-1 + q(ch0) inline; rest filled ----
        for ch in range(2):
            qk_unit(0, ch, 1)()
            for tt in range(4):
                v_unit(0, ch, tt)()
        qk_unit(0, 0, 0)()
        # k/v ch2-3 + remaining q(b0) + all of P1(b1) as fillers
        for ch in range(2, 4):
            fillers.append(qk_unit(0, ch, 1))
            for tt in range(4):
                fillers.append(v_unit(0, ch, tt))
        for ch in range(1, 4):
            fillers.append(qk_unit(0, ch, 0))
        for ch in range(4):
            fillers.append(qk_unit(1, ch, 1))
            for tt in range(4):
                fillers.append(v_unit(1, ch, tt))
        for ch in range(4):
            fillers.append(qk_unit(1, ch, 0))

        # ---------------- attention loops ----------------
        def _av(nc, accs, vab, prs, s, last):
            for h in range(2):
                for blk in range(2):
                    ki = 2 * s + blk
                    nc.tensor.matmul(
                        accs[h][:],
                        vab[ki][:, h, :],
                        prs[h][:, blk * 512 : (blk + 1) * 512],
                        start=(ki == 0),
                        stop=(ki == 15),
                    )

        for b in range(B):
            for qc in range(4):
                qsl = slice(qc * 512, (qc + 1) * 512)
                accs = {
                    h: ps.tile([128, 512], F32, tag="av", bufs=2, name=f"av{b}{qc}{h}")
                    for h in range(2)
                }
                prev = None
                for s in range(8):
                    scs, prs = {}, {}
                    for h in range(2):
                        scs[h] = ps.tile(
                            [128, 1024], F32, tag=f"sc{h}", bufs=1, name="scps"
                        )
                    for h in range(2):
                        p0 = h * 64
                        for blk in range(2):
                            ki = 2 * s + blk
                            nc.tensor.matmul(
                                scs[h][:, blk * 512 : (blk + 1) * 512],
                                kT[b][p0 : p0 + 64, ki * 128 : (ki + 1) * 128],
                                qT[b][p0 : p0 + 64, qsl],
                                start=True,
                                stop=True,
                            )
                    for h in range(2):
                        prs[h] = sb.tile(
                            [128, 1024], BF16, tag=f"pr{h}", bufs=4, name="pr"
                        )
                        nc.scalar.activation(prs[h][:], scs[h][:], EXP, scale=SCALE)
                    if prev is not None:
                        _av(nc, accs, va3[b], prev[0], prev[1], False)
                    prev = (prs, s)
                    pump(2 if (b == 0 and qc == 0) else 1)
                _av(nc, accs, va3[b], prev[0], prev[1], True)
                # ---- burst: acc rows 64-127 already hold the denominator
                # broadcast across 64 partitions (ones block in va, free via
                # the column-bound AV matmul): reciprocal and normalize
                # straight out of PSUM — no evacuation, no broadcast matmul.
                pump(2)
                for h in range(2):
                    p0 = h * 64
                    rb = sb.tile([64, 512], F32, tag="recbc", bufs=4, name="rb")
                    nc.vector.reciprocal_approx_fast(rb[:], accs[h][0:64, :])
                    nc.vector.tensor_mul(
                        oT[b][p0 : p0 + 64, qsl], accs[h][64:128, :], rb[:]
                    )
                late = True
                for j in range(4):
                    fillers.append(op_unit(b, qc * 4 + j, dve_only=late))
                pump(4 if (b == 1 and qc == 3) else 2)

        # ---------------- tail: drain remaining fillers ----------------
        while fillers:
            fillers.popleft()()

    nc.finalize()
    return nc


_NC_CACHE = None
TRACE = False  # set True (e.g. from test.py) to capture an NTFF profile
LAST_RESULT = None  # BassKernelResults of the most recent run


def _get_nc():
    global _NC_CACHE
    if _NC_CACHE is None:
        _NC_CACHE = build_kernel()
    return _NC_CACHE


def kernel(x, W_qkv, W_out, b_out):
    import ml_dtypes

    x = np.asarray(x, dtype=np.float32)
    W_qkv = np.asarray(W_qkv, dtype=np.float32)
    W_out = np.asarray(W_out, dtype=np.float32)
    b_out = np.asarray(b_out, dtype=np.float32)

    xT = np.ascontiguousarray(x.reshape(T, D).T).astype(ml_dtypes.bfloat16)
    in_maps = []
    for c in range(N_CORES):
        h0 = c * HEADS_PER_CORE
        rows = slice(h0 * HD, (h0 + 2) * HD)  # this core's 128 head dims
        wq = W_qkv[0 * D :][rows]  # [128, D]
        wk = W_qkv[1 * D :][rows]
        wv = W_qkv[2 * D :][rows]
        wqkvT = np.ascontiguousarray(np.concatenate([wq, wk, wv], axis=0).T).astype(
            ml_dtypes.bfloat16
        )
        woutT = np.ascontiguousarray(W_out[:, h0 * HD : (h0 + 2) * HD].T).astype(
            ml_dtypes.bfloat16
        )
        in_maps.append({"xT": xT, "wqkvT": wqkvT, "woutT": woutT})

    nc = _get_nc()
    global LAST_RESULT
    try:
        res = run_bass_kernel_spmd(
            nc, in_maps, core_ids=list(range(N_CORES)), trace=TRACE
        )
    except Exception:
        # transient NRT_EXEC_UNIT_UNRECOVERABLE wedges recover on retry
        # (NEURON_RT_RESET_CORES=1 re-resets the cores)
        res = run_bass_kernel_spmd(
            nc, in_maps, core_ids=list(range(N_CORES)), trace=TRACE
        )
    LAST_RESULT = res
    partial = np.zeros((T, D), dtype=np.float64)
    for c in range(N_CORES):
        partial += res.results[c]["out"].astype(np.float64)
    full = (partial + b_out.astype(np.float64)).astype(np.float32)
    return full.reshape(B, S, D)


# revision 45
# speedup vs baseline: 1.1988x; 1.1988x over previous
"""Multi-head self-attention (B=2, S=2048, D=1024, H=16) on 8 TRN2 NeuronCores.

Sharding: batch*heads tensor-parallel. Each core owns 2 heads (both batches):
QKV projection for its heads only (W_qkv output-dim sharded), full attention
for its 2x2 (batch, head) pairs, partial output projection (W_out input-dim
sharded). The 8 bf16 partial outputs are summed in fp64 on the host (the
"all-reduce"), plus the output bias.

Single fused pipeline, PE-saturated (~90% tensor-engine occupancy):
  - All matmuls bf16 (FWL fast-weight-load); x ships pre-transposed bf16
    (xT [D, T]); per-core weight slices pre-transposed on host.
  - q,k head-major (qT/kT [hd 128, tok]). v is produced DIRECTLY token-major
    by swapping matmul operands (lhsT = x tile, rhs = Wv) so no PE
    transposes; v tiles carry a LEADING 64-wide ones block per head
    (va [128, 2, 128] = [ones | v]) so each AV matmul accumulates the
    softmax denominator already BROADCAST across psum rows 0..63 (output
    in rows 64..127) — free, since matmul cost is column-count-bound.
  - attention runs in 8 loops of (batch, 512-token q-chunk), 8 k-steps each.
    Scores are k-token-major with TWO k-tiles packed per psum tile
    [128, 1024] (ki pair = column halves): exp stays 1024-wide (ACT
    efficiency) while the AV accumulators stay [128, 512] x 2 heads
    = 2 psum banks. No max-subtraction (|score*scale| < ~6 here).
    PSUM budget: scores 2x2 banks + accs 2 banks + 2 "fill" banks.
  - The 2 fill banks host everything else via tag round-robin: QKV
    projection accumulators, output-projection tiles, and the normalization
    broadcast — emitted as FILLER closures pumped between k-steps so the PE
    queue never drains: batch 1's projections run inside batch 0's attention
    loops, output projections run inside later loops. ACT runs exp plus
    the QKV-projection evacuations (early, where it idles); output
    evacuations are DVE.
  - normalization per (loop, head): reciprocal_approx_fast straight on the
    psum denominator block (rows 0..63 — the custom op corrupts on psum
    reads at base partition 64 and NaNs on 1-partition rows), one DVE
    multiply vs psum o-rows -> oT bf16. No evacuation, no broadcast matmul.
  - first x chunk DMA'd in t-quarters so the first projection matmuls start
    as soon as the first 256KB lands; NEURON_RT_RESET_CORES=1 restores
    nominal clocks if a previous run degraded the device power state.
"""

import os
import sys

# Reset cores at runtime init: restores nominal clocks if a previous run
# left the device in a degraded power state (observed ~20% slowdown).
os.environ.setdefault("NEURON_RT_RESET_CORES", "1")

for _p in ("/opt/trn_rl_repo", "/root/.axon_site/_ro/trn_rl_repo"):
    if _p not in sys.path:
        sys.path.insert(0, _p)

from collections import deque
from contextlib import ExitStack

import numpy as np

import concourse.bacc as bacc
import concourse.bass as bass
import concourse.mybir as mybir
import concourse.tile as tile
from concourse.bass_utils import run_bass_kernel_spmd

F32 = mybir.dt.float32
F32R = mybir.dt.float32r
BF16 = mybir.dt.bfloat16

B, S, D, H = 2, 2048, 1024, 16
HD = D // H  # 64
T = B * S  # 4096 tokens
SCALE = HD**-0.5
N_CORES = 8
HEADS_PER_CORE = H // N_CORES  # 2

EXP = mybir.ActivationFunctionType.Exp


def build_kernel() -> bacc.Bacc:
    nc = bacc.Bacc(target_bir_lowering=False)
    xT = nc.dram_tensor("xT", [D, T], BF16, kind="ExternalInput")
    wqkvT = nc.dram_tensor("wqkvT", [D, 384], BF16, kind="ExternalInput")
    woutT = nc.dram_tensor("woutT", [2 * HD, D], BF16, kind="ExternalInput")
    out = nc.dram_tensor("out", [T, D], BF16, kind="ExternalOutput")

    with tile.TileContext(nc) as tc, ExitStack() as ctx:
        const = ctx.enter_context(tc.tile_pool(name="const", bufs=1))
        sb = ctx.enter_context(tc.tile_pool(name="sb", bufs=1))
        ps = ctx.enter_context(tc.tile_pool(name="ps", bufs=1, space="PSUM"))

        w_sb = const.tile([128, 8, 384], BF16)
        nc.sync.dma_start(out=w_sb, in_=wqkvT.rearrange("(t p) c -> p t c", p=128))
        wo = const.tile([2 * HD, D], BF16)
        nc.sync.dma_start(out=wo, in_=woutT[:, :])

        # ---- x prefetch: all 8 chunks (b-major so b=1 arrives during b=0) --
        x_sb = {}
        for b in range(B):
            for ch in range(4):
                t = sb.tile([128, 8, 512], BF16, tag="x", bufs=8, name=f"x{b}{ch}")
                tok0 = b * S + ch * 512
                xsrc = xT[:, tok0 : tok0 + 512].rearrange("(t p) n -> p t n", p=128)
                if b == 0 and ch == 0:
                    for tq in range(4):
                        nc.sync.dma_start(
                            out=t[:, 2 * tq : 2 * tq + 2, :],
                            in_=xsrc[:, 2 * tq : 2 * tq + 2, :],
                        )
                else:
                    nc.sync.dma_start(out=t, in_=xsrc)
                x_sb[b, ch] = t

        qT, kT, oT, va3 = {}, {}, {}, {}
        for b in range(B):
            qT[b] = sb.tile([128, S], BF16, tag="qk", bufs=4, name=f"qT{b}")
            kT[b] = sb.tile([128, S], BF16, tag="qk", bufs=4, name=f"kT{b}")
            oT[b] = sb.tile([128, S], BF16, tag="ot", bufs=2, name=f"oT{b}")
            va3[b] = [None] * 16

        # ---------------- filler machinery ----------------
        fillers = deque()

        def pump(n):
            c = 0
            while fillers and c < n:
                fillers.popleft()()
                c += 1

        def qk_unit(b, ch, g):
            """QKV projection for q (g=0) or k (g=1): 8 accumulating matmuls
            + DVE evacuation into qT/kT."""

            def go():
                acc = ps.tile([128, 512], F32, tag="fill", bufs=2, name="qkacc")
                xs = x_sb[b, ch]
                for t in range(8):
                    nc.tensor.matmul(
                        acc[:],
                        w_sb[:, t, g * 128 : (g + 1) * 128],
                        xs[:, t, :],
                        start=(t == 0),
                        stop=(t == 7),
                    )
                dst = qT[b] if g == 0 else kT[b]
                nc.scalar.copy(dst[:, ch * 512 : (ch + 1) * 512], acc[:])

            return go

        def v_unit(b, ch, tt):
            """v projection for one 128-token tile, produced token-major
            (lhsT = x chunk, rhs = Wv), then augmented with ones columns."""

            def go():
                vt = ps.tile([128, 2, 64], F32, tag="fill", bufs=2, name="vt")
                xs = x_sb[b, ch]
                for t in range(8):
                    nc.tensor.matmul(
                        vt[:, :, :],
                        xs[:, t, tt * 128 : (tt + 1) * 128],
                        w_sb[:, t, 256:384],
                        start=(t == 0),
                        stop=(t == 7),
                    )
                va = sb.tile([128, 2, 128], BF16, tag="vaug", bufs=32, name="va")
                nc.vector.memset(va[:, :, 0:64], 1.0)
                nc.vector.tensor_copy(va[:, :, 64:128], vt[:, :, :])
                va3[b][ch * 4 + tt] = va

            return go

        def op_unit(b, tc_i, dve_only=False):
            """Output projection for one 128-token chunk: 2 matmuls (dout
            halves) + evac (DVE, or split DVE/ACT when ACT has slack) + DMA."""

            def go():
                for nk in range(2):
                    op = ps.tile([128, 512], F32, tag="fill", bufs=2, name="opps")
                    nc.tensor.matmul(
                        op[:],
                        oT[b][:, tc_i * 128 : (tc_i + 1) * 128],
                        wo[:, nk * 512 : (nk + 1) * 512],
                        start=True,
                        stop=True,
                    )
                    ob = sb.tile([128, 512], BF16, tag="outsb", bufs=10, name="ob")
                    if nk == 0 or dve_only:
                        nc.vector.tensor_copy(ob[:], op[:])
                    else:
                        nc.scalar.copy(ob[:], op[:])
                    r0 = b * S + tc_i * 128
                    nc.sync.dma_start(
                        out=out[r0 : r0 + 128, nk * 512 : (nk + 1) * 512], in_=ob[:]
                    )

            return go

        # ---------------- P1(b0): k/v ch0-1 + q(ch0) inline; rest filled ----
        for ch in range(2):
            qk_unit(0, ch, 1)()
            for tt in range(4):
                v_unit(0, ch, tt)()
        qk_unit(0, 0, 0)()
        # k/v ch2-3 + remaining q(b0) + all of P1(b1) as fillers
        for ch in range(2, 4):
            fillers.append(qk_unit(0, ch, 1))
            for tt in range(4):
                fillers.append(v_unit(0, ch, tt))
        for ch in range(1, 4):
            fillers.append(qk_unit(0, ch, 0))
        for ch in range(4):
            fillers.append(qk_unit(1, ch, 1))
            for tt in range(4):
                fillers.append(v_unit(1, ch, tt))
        for ch in range(4):
            fillers.append(qk_unit(1, ch, 0))

        # ---------------- attention loops ----------------
        def _av(nc, accs, vab, prs, s, last):
            for h in range(2):
                for blk in range(2):
                    ki = 2 * s + blk
                    nc.tensor.matmul(
                        accs[h][:],
                        vab[ki][:, h, :],
                        prs[h][:, blk * 512 : (blk + 1) * 512],
                        start=(ki == 0),
                        stop=(ki == 15),
                    )

        for b in range(B):
            for qc in range(4):
                qsl = slice(qc * 512, (qc + 1) * 512)
                accs = {
                    h: ps.tile([128, 512], F32, tag="av", bufs=2, name=f"av{b}{qc}{h}")
                    for h in range(2)
                }
                prev = None
                for s in range(8):
                    scs, prs = {}, {}
                    for h in range(2):
                        scs[h] = ps.tile(
                            [128, 1024], F32, tag=f"sc{h}", bufs=1, name="scps"
                        )
                    for h in range(2):
                        p0 = h * 64
                        for blk in range(2):
                            ki = 2 * s + blk
                            nc.tensor.matmul(
                                scs[h][:, blk * 512 : (blk + 1) * 512],
                                kT[b][p0 : p0 + 64, ki * 128 : (ki + 1) * 128],
                                qT[b][p0 : p0 + 64, qsl],
                                start=True,
                                stop=True,
                            )
                    for h in range(2):
                        prs[h] = sb.tile(
                            [128, 1024], BF16, tag=f"pr{h}", bufs=4, name="pr"
                        )
                        nc.scalar.activation(prs[h][:], scs[h][:], EXP, scale=SCALE)
                    if prev is not None:
                        _av(nc, accs, va3[b], prev[0], prev[1], False)
                    prev = (prs, s)
                    pump(2 if (b == 0 and qc == 0) else 1)
                _av(nc, accs, va3[b], prev[0], prev[1], True)
                # ---- burst: acc rows 64-127 already hold the denominator
                # broadcast across 64 partitions (ones block in va, free via
                # the column-bound AV matmul): reciprocal and normalize
                # straight out of PSUM — no evacuation, no broadcast matmul.
                pump(1)
                for h in range(2):
                    p0 = h * 64
                    rb = sb.tile([64, 512], F32, tag="recbc", bufs=2, name="rb")
                    nc.vector.reciprocal_approx_fast(rb[:], accs[h][0:64, :])
                    nc.vector.tensor_mul(
                        oT[b][p0 : p0 + 64, qsl], accs[h][64:128, :], rb[:]
                    )
                late = True
                for j in range(4):
                    fillers.append(op_unit(b, qc * 4 + j, dve_only=late))
                pump(4 if (b == 1 and qc == 3) else 2)

        # ---------------- tail: drain remaining fillers ----------------
        while fillers:
            fillers.popleft()()

    nc.finalize()
    return nc


_NC_CACHE = None
TRACE = False  # set True (e.g. from test.py) to capture an NTFF profile
LAST_RESULT = None  # BassKernelResults of the most recent run


def _get_nc():
    global _NC_CACHE
    if _NC_CACHE is None:
        _NC_CACHE = build_kernel()
    return _NC_CACHE


def kernel(x, W_qkv, W_out, b_out):
    import ml_dtypes

    x = np.asarray(x, dtype=np.float32)
    W_qkv = np.asarray(W_qkv, dtype=np.float32)
    W_out = np.asarray(W_out, dtype=np.float32)
    b_out = np.asarray(b_out, dtype=np.float32)

    xT = np.ascontiguousarray(x.reshape(T, D).T).astype(ml_dtypes.bfloat16)
    in_maps = []
    for c in range(N_CORES):
        h0 = c * HEADS_PER_CORE
        rows = slice(h0 * HD, (h0 + 2) * HD)  # this core's 128 head dims
        wq = W_qkv[0 * D :][rows]  # [128, D]
        wk = W_qkv[1 * D :][rows]
        wv = W_qkv[2 * D :][rows]
        wqkvT = np.ascontiguousarray(np.concatenate([wq, wk, wv], axis=0).T).astype(
            ml_dtypes.bfloat16
        )
        woutT = np.ascontiguousarray(W_out[:, h0 * HD : (h0 + 2) * HD].T).astype(
            ml_dtypes.bfloat16
        )
        in_maps.append({"xT": xT, "wqkvT": wqkvT, "woutT": woutT})

    nc = _get_nc()
    global LAST_RESULT
    try:
        res = run_bass_kernel_spmd(
            nc, in_maps, core_ids=list(range(N_CORES)), trace=TRACE
        )
    except Exception:
        # transient NRT_EXEC_UNIT_UNRECOVERABLE wedges recover on retry
        # (NEURON_RT_RESET_CORES=1 re-resets the cores)
        res = run_bass_kernel_spmd(
            nc, in_maps, core_ids=list(range(N_CORES)), trace=TRACE
        )
    LAST_RESULT = res
    partial = np.zeros((T, D), dtype=np.float64)
    for c in range(N_CORES):
        partial += res.results[c]["out"].astype(np.float64)
    full = (partial + b_out.astype(np.float64)).astype(np.float32)
    return full.reshape(B, S, D)
